# revision 37
# baseline (speedup 1.0000x reference)
"""Trainium2 Bass kernel for GQA attention (B=2, S=2048, D=2048, 16 q-heads,
4 kv-heads, head_dim=128, RoPE, causal) sharded over 8 NeuronCores.

Sharding: core c handles batch b = c//4 and q-head group g = c%4
(q-heads 4g..4g+3, which share kv-head g).  Each core computes a partial
output o_part[b] = sum_{its heads} attn_head @ Wo_head; the host sums the
4 partials per batch.
"""

import sys

sys.path.insert(0, "/opt/trn_rl_repo")

import math

import numpy as np

P = 128
NEG = -1.0e9
EXP_BIAS = -8.0  # exp(s - 8): cancels in softmax normalization, avoids overflow


def build_nc(S=2048, D=2048, QH=4, H=128, theta=10000.0):
    """Build the per-core Bass graph.

    Per-core problem: x [S, D] f32, positions [S] i32,
    wq [QH, D, H] f32 (pre-scaled by 1/sqrt(H)), wk/wv [D, H] f32,
    wo [QH, H, D] f32  ->  o [S, D] f32 (partial over heads).
    """
    import concourse.bacc as bacc
    import concourse.mybir as mybir
    from concourse import tile
    from concourse.masks import make_identity

    f32 = mybir.dt.float32
    bf16 = mybir.dt.bfloat16
    i32 = mybir.dt.int32
    ADD = mybir.AluOpType.add
    MULT = mybir.AluOpType.mult
    EXP = mybir.ActivationFunctionType.Exp
    SIN = mybir.ActivationFunctionType.Sin

    assert H == P
    HH = H // 2  # 64
    DK = D // P  # d-chunks
    NSQ = S // P  # s-tiles
    SB = min(512, S)  # sq block width
    NSB = S // SB  # sq blocks
    RB = SB // P  # sq subtiles per block
    NT = S // P  # t tiles
    TRG = min(512, D)  # transpose psum group width
    CS = min(256, S)  # rope chunk width

    nc = bacc.Bacc(None, target_bir_lowering=False)

    x_d = nc.declare_dram_parameter("x", [S, D], f32, isOutput=False)
    pos_d = nc.declare_dram_parameter("positions", [S], i32, isOutput=False)
    wq_d = nc.declare_dram_parameter("wq", [QH, D, H], f32, isOutput=False)
    wk_d = nc.declare_dram_parameter("wk", [D, H], f32, isOutput=False)
    wv_d = nc.declare_dram_parameter("wv", [D, H], f32, isOutput=False)
    wo_d = nc.declare_dram_parameter("wo", [QH, H, D], f32, isOutput=False)
    o_d = nc.declare_dram_parameter("o", [S, D], f32, isOutput=True)

    from contextlib import ExitStack

    with tile.TileContext(nc) as tc, ExitStack() as es:
        # ---------------- pools ----------------
        const = es.enter_context(tc.tile_pool(name="const", bufs=1))
        stage = es.enter_context(tc.tile_pool(name="stage", bufs=2))
        persist = es.enter_context(tc.tile_pool(name="persist", bufs=1))
        small = es.enter_context(tc.tile_pool(name="small", bufs=2))
        pt_pool = es.enter_context(tc.tile_pool(name="pt", bufs=6))
        ob_pool = es.enter_context(tc.tile_pool(name="ob", bufs=2))
        at_pool = es.enter_context(tc.tile_pool(name="at", bufs=1))
        # PSUM: "sc" (x-transpose groups, scores, O proj) 3 banks,
        # ptr2 (attn transpose) 1 bank, AV accumulators 4 banks.
        ps_sc = es.enter_context(tc.tile_pool(name="ps_sc", bufs=4, space="PSUM"))
        ps_av = es.enter_context(tc.tile_pool(name="ps_av", bufs=1, space="PSUM"))

        # ---------------- constants ----------------
        identf = const.tile([P, P], f32)
        make_identity(nc, identf)
        ident = const.tile([P, P], bf16)
        make_identity(nc, ident)

        exp_bias = const.tile([P, 1], f32)
        nc.gpsimd.memset(exp_bias[:], EXP_BIAS)

        # causal additive mask for the diagonal [P, P] sub-block of a
        # scoresT tile: keep (0) where y >= x, else NEG.
        mask = const.tile([P, P], f32)
        nc.gpsimd.memset(mask[:], 0.0)
        nc.gpsimd.affine_select(
            out=mask[:],
            in_=mask[:],
            compare_op=mybir.AluOpType.is_ge,
            fill=NEG,
            base=0,
            pattern=[[1, P]],
            channel_multiplier=-1,
        )

        # ---------------- rope tables (emitted first: DVE chain runs
        # while x DMAs stream on the sync queue) ----------------
        # inv_ts[i] = theta ** (-2 i / H), i in [0, HH)
        iot = const.tile([HH, 1], i32)
        nc.gpsimd.iota(iot[:], pattern=[[0, 1]], base=0, channel_multiplier=1)
        iotf = const.tile([HH, 1], f32)
        nc.vector.tensor_copy(iotf[:], iot[:])
        inv_ts = const.tile([HH, 1], f32)
        nc.scalar.activation(
            inv_ts[:], iotf[:], EXP, scale=-2.0 * math.log(theta) / H
        )

        TWO_PI = float(np.float32(2.0 * math.pi))
        PI = float(np.float32(math.pi))

        # cos2[h] = cos(angle_{h mod HH}); sin2s[h<HH] = -sin, sin2s[h>=HH] = +sin
        cos2 = persist.tile([P, S], f32)
        sin2s = persist.tile([P, S], f32)

        for c0 in range(0, S, CS):
            sl = slice(c0, c0 + CS)
            posi = const.tile([1, CS], i32, tag="rr_pi", name="posi")
            nc.gpsimd.dma_start(
                posi[:], pos_d.rearrange("(a s) -> a s", a=1)[:, sl]
            )
            posf = const.tile([1, CS], f32, tag="rr_pf", name="posf")
            nc.vector.tensor_copy(posf[:], posi[:])
            pb = const.tile([HH, CS], f32, tag="rr_pb", name="pb")
            nc.gpsimd.partition_broadcast(pb[:], posf[:])
            ang = const.tile([HH, CS], f32, tag="rr_ang", name="ang")
            nc.vector.tensor_scalar_mul(ang[:], pb[:], inv_ts[:])

            def sin_reduced(dst, phase):
                # dst = sin(ang + phase).  k = int-cast((ang+phase)/2pi):
                # trunc (sim) gives red in [0, 2pi); round (hw) gives
                # [-pi, pi].  One conditional -2pi brings both to [-pi, pi].
                if phase != 0.0:
                    a = const.tile([HH, CS], f32, tag="rr_a", name="a", bufs=1)
                    nc.vector.tensor_scalar_add(a[:], ang[:], phase)
                else:
                    a = ang
                t = const.tile([HH, CS], f32, tag="rr_t", name="t", bufs=1)
                nc.vector.tensor_scalar_mul(t[:], a[:], 1.0 / TWO_PI)
                ki = const.tile([HH, CS], i32, tag="rr_ki", name="ki", bufs=1)
                nc.vector.tensor_copy(ki[:], t[:])
                kf = const.tile([HH, CS], f32, tag="rr_kf", name="kf", bufs=1)
                nc.vector.tensor_copy(kf[:], ki[:])
                red = const.tile([HH, CS], f32, tag="rr_red", name="red", bufs=1)
                nc.vector.scalar_tensor_tensor(
                    red[:], kf[:], -TWO_PI, a[:], MULT, ADD
                )
                cc = const.tile([HH, CS], f32, tag="rr_c", name="cc", bufs=1)
                nc.vector.tensor_scalar(
                    cc[:], red[:], PI, None, op0=mybir.AluOpType.is_gt
                )
                nc.vector.scalar_tensor_tensor(
                    red[:], cc[:], -TWO_PI, red[:], MULT, ADD
                )
                nc.scalar.activation(dst[:], red[:], SIN)

            sin_reduced(cos2[0:HH, sl], float(np.float32(math.pi / 2.0)))
            sin_reduced(sin2s[HH:P, sl], 0.0)  # +sin in hi half

        nc.vector.tensor_copy(cos2[HH:P, :], cos2[0:HH, :])
        nc.vector.tensor_scalar_mul(sin2s[0:HH, :], sin2s[HH:P, :], -1.0)

        # PE warm-up burst: ~9us of dense matmuls while the first x DMA is
        # in flight, so the HAM clock gate opens before real work arrives.
        for wu in range(10):
            pwu = ps_sc.tile([P, 512], f32, tag="sc", name="pwu")
            for j in range(4):
                nc.tensor.matmul(
                    pwu[:, j * P : (j + 1) * P],
                    identf[:],
                    identf[:],
                    start=True,
                    stop=True,
                )

        # ---------------- x load + transpose (f32 in PE, cast on evict) --
        # xT layout [p, st, dk*P + u] : element x(s = st*P + u, d = dk*P + p)
        xT = persist.tile([P, NSQ, D], bf16)
        for st in range(NSQ):
            xf = stage.tile([P, D], f32, tag="xf", name="xf", bufs=3)
            for g in range(D // TRG):
                nc.sync.dma_start(
                    xf[:, g * TRG : (g + 1) * TRG],
                    x_d[st * P : (st + 1) * P, g * TRG : (g + 1) * TRG],
                )
            for g in range(D // TRG):
                ptr = ps_sc.tile([P, TRG], f32, tag="sc", name="ptr")
                for j in range(TRG // P):
                    dk = (TRG // P) * g + j
                    nc.tensor.transpose(
                        ptr[:, j * P : (j + 1) * P],
                        xf[:, dk * P : (dk + 1) * P],
                        identf[:],
                    )
                if (st + g) % 2 == 0:
                    nc.scalar.copy(xT[:, st, g * TRG : (g + 1) * TRG], ptr[:])
                else:
                    nc.vector.tensor_copy(
                        xT[:, st, g * TRG : (g + 1) * TRG], ptr[:]
                    )

        # ---------------- weights: load f32, cast to bf16 ----------------
        # layout [p, dk, h]: element (d = dk*P + p, h)
        def load_w_dh(dram_ap, name):  # dram [D, H] -> sbuf bf16 [P, DK, H]
            wf = stage.tile([P, DK * H], f32, tag="xf", name="wf", bufs=3)
            nc.gpsimd.dma_start(
                wf[:].rearrange("p (k h) -> p k h", k=DK),
                dram_ap.rearrange("(k p) h -> p k h", p=P),
            )
            wb = persist.tile([P, DK, H], bf16, name=name, tag=name)
            nc.scalar.copy(wb[:], wf[:].rearrange("p (k h) -> p k h", k=DK))
            return wb

        wq_sb = [load_w_dh(wq_d[h], f"wq{h}") for h in range(QH)]
        wk_sb = load_w_dh(wk_d, "wk")
        wv_sb = load_w_dh(wv_d, "wv")

        # wo: [H, D] per head -> sbuf bf16 [P, D] (partition = h)
        wo_sb = []
        for h in range(QH):
            wf = stage.tile([P, D], f32, tag="xf", name="wf", bufs=3)
            nc.gpsimd.dma_start(wf[:], wo_d[h])
            wb = persist.tile([P, D], bf16, name=f"wo{h}", tag=f"wo{h}")
            nc.scalar.copy(wb[:], wf[:])
            wo_sb.append(wb)

        # ---------------- q/k projections with rope ----------------
        def proj_qk(w_sb, out_tile):
            for sb in range(NSB):
                pq = ps_sc.tile([P, SB], f32, tag="sc", name="pq")
                for dk in range(DK):
                    nc.tensor.matmul(
                        pq[:],
                        w_sb[:, dk, :],
                        xT[:, sb * RB : (sb + 1) * RB, dk * P : (dk + 1) * P],
                        start=(dk == 0),
                        stop=(dk == DK - 1),
                    )
                sl = slice(sb * SB, (sb + 1) * SB)
                # rope: out = pq * cos2 + rot(pq) * sin2s
                tsin = small.tile([P, SB], f32, tag="tsin")
                nc.vector.tensor_tensor(
                    tsin[0:HH, :], pq[HH:P, :], sin2s[0:HH, sl], MULT
                )
                nc.vector.tensor_tensor(
                    tsin[HH:P, :], pq[0:HH, :], sin2s[HH:P, sl], MULT
                )
                tcos = small.tile([P, SB], f32, tag="tcos")
                nc.vector.tensor_tensor(tcos[:], pq[:], cos2[:, sl], MULT)
                nc.vector.tensor_tensor(out_tile[:, sl], tcos[:], tsin[:], ADD)

        qT = [persist.tile([P, S], bf16, name=f"qT{h}", tag=f"qT{h}") for h in range(QH)]
        kT = persist.tile([P, S], bf16)
        for h in range(QH):
            proj_qk(wq_sb[h], qT[h])
        proj_qk(wk_sb, kT)

        # ---------------- v projection (v' with ones column) -------------
        # layout [P, NT, H+4]: v[t = tt*P + p, 0:H], v'[t, H] = 1
        VW = H + 4
        vp = persist.tile([P, NT, VW], bf16)
        for tt in range(NT):
            pv = ps_sc.tile([P, P], f32, tag="sc", name="pv")
            for dk in range(DK):
                nc.tensor.matmul(
                    pv[:],
                    xT[:, tt, dk * P : (dk + 1) * P],
                    wv_sb[:, dk, :],
                    start=(dk == 0),
                    stop=(dk == DK - 1),
                )
            nc.vector.tensor_copy(vp[:, tt, 0:H], pv[:])
            nc.gpsimd.memset(vp[:, tt, H : H + 1], 1.0)

        # ---------------- attention + fused O projection, per sq block ----
        # O-projection of block sb-1 is interleaved between the attention
        # heads of block sb so its PSUM-evict waits don't stall the PE queue.
        def oproj_tile(sb, attnT_blk, r2):
            st = RB * sb + r2
            for db in range(D // SB):
                po = ps_sc.tile([P, SB], f32, tag="sc", name="po")
                for h in range(QH):
                    nc.tensor.matmul(
                        po[:],
                        attnT_blk[h][:, r2 * P : (r2 + 1) * P],
                        wo_sb[h][:, db * SB : (db + 1) * SB],
                        start=(h == 0),
                        stop=(h == QH - 1),
                    )
                ob = ob_pool.tile([P, SB], f32, tag="ob")
                nc.vector.tensor_copy(ob[:], po[:])
                nc.sync.dma_start(
                    o_d[st * P : (st + 1) * P, db * SB : (db + 1) * SB], ob[:]
                )

        def attention_head(sb, h, attnT):
            pav = [
                ps_av.tile(
                    [P, H + 1], f32, name=f"pav{r}", tag=f"av{r}", bufs=1
                )[:]
                for r in range(RB)
            ]
            ptr2 = ps_sc.tile([P, SB], bf16, tag="sc", name="ptr2")
            ans = [None] * RB

            def finish_subtile(r2):
                rec = small.tile([P, 1], f32, tag="rec", bufs=4)
                nc.vector.reciprocal(rec[:], pav[r2][:, H : H + 1])
                an = small.tile([P, H], bf16, tag="an", bufs=4)
                nc.vector.tensor_scalar_mul(an[:], pav[r2][:, 0:H], rec[:])
                ans[r2] = an

            for tt in range(RB * (sb + 1)):
                pscore = ps_sc.tile([P, SB], f32, tag="sc", name="pscore")
                nc.tensor.matmul(
                    pscore[:],
                    kT[:, tt * P : (tt + 1) * P],
                    qT[h][:, sb * SB : (sb + 1) * SB],
                    start=True,
                    stop=True,
                )
                r = tt - RB * sb
                if r >= 0:
                    nc.vector.tensor_tensor(
                        pscore[:, r * P : (r + 1) * P],
                        pscore[:, r * P : (r + 1) * P],
                        mask[:],
                        ADD,
                    )
                pt = pt_pool.tile([P, SB], bf16, tag="pt")
                c0 = max(0, r) * P
                nc.scalar.activation(
                    pt[:, c0:SB], pscore[:, c0:SB], EXP, bias=exp_bias[:]
                )
                for r2 in range(max(0, r), RB):
                    q128 = RB * sb + r2
                    nc.tensor.matmul(
                        pav[r2],
                        pt[:, r2 * P : (r2 + 1) * P],
                        vp[:, tt, 0 : H + 1],
                        start=(tt == 0),
                        stop=(tt == q128),
                    )
                if r >= 0:
                    finish_subtile(r)
            for r2 in range(RB):
                nc.tensor.transpose(
                    ptr2[:, r2 * P : (r2 + 1) * P], ans[r2][:], ident[:]
                )
                sl2 = slice(r2 * P, (r2 + 1) * P)
                nc.scalar.copy(attnT[h][:, sl2], ptr2[:, sl2])

        prev = None
        for sb in range(NSB):
            attnT = [
                at_pool.tile(
                    [P, SB], bf16, name=f"attnT{h}", tag=f"attnT{h}", bufs=2
                )
                for h in range(QH)
            ]
            for h in range(QH):
                attention_head(sb, h, attnT)
                if prev is not None:
                    oproj_tile(sb - 1, prev, h)
            prev = attnT
        for r2 in range(RB):
            oproj_tile(NSB - 1, prev, r2)

    nc.compile()
    return nc


_NC_CACHE = {}


def _get_nc(key):
    if key not in _NC_CACHE:
        _NC_CACHE[key] = build_nc(*key)
    return _NC_CACHE[key]


def make_in_maps(x, positions, Wq, Wk, Wv, Wo, n_cores=8):
    B, S, D = x.shape
    Q, _, H = Wq.shape
    N = Wk.shape[0]
    groups = Q // N if N else 1
    gpb = n_cores // B  # head groups per batch (4)
    qh_per_core = Q // gpb
    assert qh_per_core * gpb == Q
    scale = np.float32(1.0 / math.sqrt(H))
    in_maps = []
    for c in range(n_cores):
        b = c // gpb
        g = c % gpb
        qh0 = g * qh_per_core
        kvh = qh0 // groups
        in_maps.append(
            {
                "x": np.ascontiguousarray(x[b]),
                "positions": positions,
                "wq": np.ascontiguousarray(Wq[qh0 : qh0 + qh_per_core] * scale),
                "wk": np.ascontiguousarray(Wk[kvh]),
                "wv": np.ascontiguousarray(Wv[kvh]),
                "wo": np.ascontiguousarray(Wo[qh0 : qh0 + qh_per_core]),
            }
        )
    return in_maps, gpb, qh_per_core


def kernel(x, positions, Wq, Wk, Wv, Wo):
    """Full inputs -> full output.  x [B,S,D] f32, positions [S] i32,
    Wq [Q,D,H], Wk/Wv [N,D,H], Wo [Q,H,D].  Returns [B,S,D] f32."""
    from concourse.bass_utils import run_bass_kernel_spmd

    x = np.ascontiguousarray(np.asarray(x, dtype=np.float32))
    positions = np.ascontiguousarray(np.asarray(positions, dtype=np.int32))
    Wq = np.asarray(Wq, dtype=np.float32)
    Wk = np.asarray(Wk, dtype=np.float32)
    Wv = np.asarray(Wv, dtype=np.float32)
    Wo = np.asarray(Wo, dtype=np.float32)

    B, S, D = x.shape
    Q, _, H = Wq.shape
    n_cores = 8
    in_maps, gpb, qh_per_core = make_in_maps(x, positions, Wq, Wk, Wv, Wo, n_cores)

    nc = _get_nc((S, D, qh_per_core, H))
    res = run_bass_kernel_spmd(nc, in_maps, core_ids=list(range(n_cores)))
    out = np.zeros((B, S, D), dtype=np.float32)
    for c in range(n_cores):
        out[c // gpb] += res.results[c]["o"]
    return out


# revision 38
# speedup vs baseline: 1.1640x; 1.1640x over previous
"""Trainium2 Bass kernel for GQA attention (B=2, S=2048, D=2048, 16 q-heads,
4 kv-heads, head_dim=128, RoPE, causal) sharded over 8 NeuronCores.

Sharding: core c handles batch b = c//4 and q-head group g = c%4
(q-heads 4g..4g+3, which share kv-head g).  Each core computes a partial
output o_part[b] = sum_{its heads} attn_head @ Wo_head; the host sums the
4 partials per batch.
"""

import sys

sys.path.insert(0, "/opt/trn_rl_repo")

import math

import numpy as np

P = 128
NEG = -1.0e9
EXP_BIAS = -8.0  # exp(s - 8): cancels in softmax normalization, avoids overflow


def build_nc(S=2048, D=2048, QH=4, H=128, theta=10000.0):
    """Build the per-core Bass graph.

    Per-core problem: x [S, D] f32, positions [S] i32,
    wq [QH, D, H] f32 (pre-scaled by 1/sqrt(H)), wk/wv [D, H] f32,
    wo [QH, H, D] f32  ->  o [S, D] f32 (partial over heads).
    """
    import concourse.bacc as bacc
    import concourse.mybir as mybir
    from concourse import tile
    from concourse.masks import make_identity

    f32 = mybir.dt.float32
    bf16 = mybir.dt.bfloat16
    i32 = mybir.dt.int32
    ADD = mybir.AluOpType.add
    MULT = mybir.AluOpType.mult
    EXP = mybir.ActivationFunctionType.Exp
    SIN = mybir.ActivationFunctionType.Sin

    assert H == P
    HH = H // 2  # 64
    DK = D // P  # d-chunks
    NSQ = S // P  # s-tiles
    SB = min(512, S)  # sq block width
    NSB = S // SB  # sq blocks
    RB = SB // P  # sq subtiles per block
    NT = S // P  # t tiles
    TRG = min(512, D)  # transpose psum group width
    CS = min(256, S)  # rope chunk width

    nc = bacc.Bacc(None, target_bir_lowering=False)

    x_d = nc.declare_dram_parameter("x", [S, D], f32, isOutput=False)
    pos_d = nc.declare_dram_parameter("positions", [S], i32, isOutput=False)
    wq_d = nc.declare_dram_parameter("wq", [QH, D, H], f32, isOutput=False)
    wk_d = nc.declare_dram_parameter("wk", [D, H], f32, isOutput=False)
    wv_d = nc.declare_dram_parameter("wv", [D, H], f32, isOutput=False)
    wo_d = nc.declare_dram_parameter("wo", [QH, H, D], f32, isOutput=False)
    o_d = nc.declare_dram_parameter("o", [S, D], f32, isOutput=True)

    from contextlib import ExitStack

    with tile.TileContext(nc) as tc, ExitStack() as es:
        # ---------------- pools ----------------
        const = es.enter_context(tc.tile_pool(name="const", bufs=1))
        stage = es.enter_context(tc.tile_pool(name="stage", bufs=2))
        persist = es.enter_context(tc.tile_pool(name="persist", bufs=1))
        small = es.enter_context(tc.tile_pool(name="small", bufs=2))
        pt_pool = es.enter_context(tc.tile_pool(name="pt", bufs=6))
        ob_pool = es.enter_context(tc.tile_pool(name="ob", bufs=2))
        at_pool = es.enter_context(tc.tile_pool(name="at", bufs=1))
        # PSUM: "sc" (x-transpose groups, scores, O proj) 3 banks,
        # ptr2 (attn transpose) 1 bank, AV accumulators 4 banks.
        ps_sc = es.enter_context(tc.tile_pool(name="ps_sc", bufs=4, space="PSUM"))
        ps_av = es.enter_context(tc.tile_pool(name="ps_av", bufs=1, space="PSUM"))

        # ---------------- constants ----------------
        identf = const.tile([P, P], f32)
        make_identity(nc, identf)
        ident = const.tile([P, P], bf16)
        make_identity(nc, ident)

        exp_bias = const.tile([P, 1], f32)
        nc.gpsimd.memset(exp_bias[:], EXP_BIAS)

        # causal additive mask for the diagonal [P, P] sub-block of a
        # scoresT tile: keep (0) where y >= x, else NEG.
        mask = const.tile([P, P], f32)
        nc.gpsimd.memset(mask[:], 0.0)
        nc.gpsimd.affine_select(
            out=mask[:],
            in_=mask[:],
            compare_op=mybir.AluOpType.is_ge,
            fill=NEG,
            base=0,
            pattern=[[1, P]],
            channel_multiplier=-1,
        )

        # ---------------- rope tables (emitted first: DVE chain runs
        # while x DMAs stream on the sync queue) ----------------
        # inv_ts[i] = theta ** (-2 i / H), i in [0, HH)
        iot = const.tile([HH, 1], i32)
        nc.gpsimd.iota(iot[:], pattern=[[0, 1]], base=0, channel_multiplier=1)
        iotf = const.tile([HH, 1], f32)
        nc.vector.tensor_copy(iotf[:], iot[:])
        inv_ts = const.tile([HH, 1], f32)
        nc.scalar.activation(
            inv_ts[:], iotf[:], EXP, scale=-2.0 * math.log(theta) / H
        )

        TWO_PI = float(np.float32(2.0 * math.pi))
        PI = float(np.float32(math.pi))

        # cos2[h] = cos(angle_{h mod HH}); sin2s[h<HH] = -sin, sin2s[h>=HH] = +sin
        cos2 = persist.tile([P, S], f32)
        sin2s = persist.tile([P, S], f32)

        for c0 in range(0, S, CS):
            sl = slice(c0, c0 + CS)
            posi = const.tile([1, CS], i32, tag="rr_pi", name="posi")
            nc.gpsimd.dma_start(
                posi[:], pos_d.rearrange("(a s) -> a s", a=1)[:, sl]
            )
            posf = const.tile([1, CS], f32, tag="rr_pf", name="posf")
            nc.vector.tensor_copy(posf[:], posi[:])
            pb = const.tile([HH, CS], f32, tag="rr_pb", name="pb")
            nc.gpsimd.partition_broadcast(pb[:], posf[:])
            ang = const.tile([HH, CS], f32, tag="rr_ang", name="ang")
            nc.vector.tensor_scalar_mul(ang[:], pb[:], inv_ts[:])

            def sin_reduced(dst, phase):
                # dst = sin(ang + phase).  k = int-cast((ang+phase)/2pi):
                # trunc (sim) gives red in [0, 2pi); round (hw) gives
                # [-pi, pi].  One conditional -2pi brings both to [-pi, pi].
                if phase != 0.0:
                    a = const.tile([HH, CS], f32, tag="rr_a", name="a", bufs=1)
                    nc.vector.tensor_scalar_add(a[:], ang[:], phase)
                else:
                    a = ang
                t = const.tile([HH, CS], f32, tag="rr_t", name="t", bufs=1)
                nc.vector.tensor_scalar_mul(t[:], a[:], 1.0 / TWO_PI)
                ki = const.tile([HH, CS], i32, tag="rr_ki", name="ki", bufs=1)
                nc.vector.tensor_copy(ki[:], t[:])
                kf = const.tile([HH, CS], f32, tag="rr_kf", name="kf", bufs=1)
                nc.vector.tensor_copy(kf[:], ki[:])
                red = const.tile([HH, CS], f32, tag="rr_red", name="red", bufs=1)
                nc.vector.scalar_tensor_tensor(
                    red[:], kf[:], -TWO_PI, a[:], MULT, ADD
                )
                cc = const.tile([HH, CS], f32, tag="rr_c", name="cc", bufs=1)
                nc.vector.tensor_scalar(
                    cc[:], red[:], PI, None, op0=mybir.AluOpType.is_gt
                )
                nc.vector.scalar_tensor_tensor(
                    red[:], cc[:], -TWO_PI, red[:], MULT, ADD
                )
                nc.scalar.activation(dst[:], red[:], SIN)

            sin_reduced(cos2[0:HH, sl], float(np.float32(math.pi / 2.0)))
            sin_reduced(sin2s[HH:P, sl], 0.0)  # +sin in hi half

        nc.vector.tensor_copy(cos2[HH:P, :], cos2[0:HH, :])
        nc.vector.tensor_scalar_mul(sin2s[0:HH, :], sin2s[HH:P, :], -1.0)

        # ---------------- x load + transpose (f32 in PE, cast on evict) --
        # xT layout [p, st, dk*P + u] : element x(s = st*P + u, d = dk*P + p)
        xT = persist.tile([P, NSQ, D], bf16)
        for st in range(NSQ):
            xf = stage.tile([P, D], f32, tag="xf", name="xf", bufs=3)
            for g in range(D // TRG):
                nc.sync.dma_start(
                    xf[:, g * TRG : (g + 1) * TRG],
                    x_d[st * P : (st + 1) * P, g * TRG : (g + 1) * TRG],
                )
            for g in range(D // TRG):
                ptr = ps_sc.tile([P, TRG], f32, tag="sc", name="ptr")
                for j in range(TRG // P):
                    dk = (TRG // P) * g + j
                    nc.tensor.transpose(
                        ptr[:, j * P : (j + 1) * P],
                        xf[:, dk * P : (dk + 1) * P],
                        identf[:],
                    )
                if (st + g) % 2 == 0:
                    nc.scalar.copy(xT[:, st, g * TRG : (g + 1) * TRG], ptr[:])
                else:
                    nc.vector.tensor_copy(
                        xT[:, st, g * TRG : (g + 1) * TRG], ptr[:]
                    )

        # ---------------- weights: load f32, cast to bf16 ----------------
        # layout [p, dk, h]: element (d = dk*P + p, h)
        def load_w_dh(dram_ap, name):  # dram [D, H] -> sbuf bf16 [P, DK, H]
            wf = stage.tile([P, DK * H], f32, tag="xf", name="wf", bufs=3)
            nc.gpsimd.dma_start(
                wf[:].rearrange("p (k h) -> p k h", k=DK),
                dram_ap.rearrange("(k p) h -> p k h", p=P),
            )
            wb = persist.tile([P, DK, H], bf16, name=name, tag=name)
            nc.scalar.copy(wb[:], wf[:].rearrange("p (k h) -> p k h", k=DK))
            return wb

        wq_sb = [load_w_dh(wq_d[h], f"wq{h}") for h in range(QH)]
        wk_sb = load_w_dh(wk_d, "wk")
        wv_sb = load_w_dh(wv_d, "wv")

        # wo: [H, D] per head -> sbuf bf16 [P, D] (partition = h)
        wo_sb = []
        for h in range(QH):
            wf = stage.tile([P, D], f32, tag="xf", name="wf", bufs=3)
            nc.gpsimd.dma_start(wf[:], wo_d[h])
            wb = persist.tile([P, D], bf16, name=f"wo{h}", tag=f"wo{h}")
            nc.scalar.copy(wb[:], wf[:])
            wo_sb.append(wb)

        # ---------------- q/k projections with rope ----------------
        def proj_qk(w_sb, out_tile):
            for sb in range(NSB):
                pq = ps_sc.tile([P, SB], f32, tag="sc", name="pq")
                for dk in range(DK):
                    nc.tensor.matmul(
                        pq[:],
                        w_sb[:, dk, :],
                        xT[:, sb * RB : (sb + 1) * RB, dk * P : (dk + 1) * P],
                        start=(dk == 0),
                        stop=(dk == DK - 1),
                    )
                sl = slice(sb * SB, (sb + 1) * SB)
                # rope: out = pq * cos2 + rot(pq) * sin2s
                tsin = small.tile([P, SB], f32, tag="tsin")
                nc.vector.tensor_tensor(
                    tsin[0:HH, :], pq[HH:P, :], sin2s[0:HH, sl], MULT
                )
                nc.vector.tensor_tensor(
                    tsin[HH:P, :], pq[0:HH, :], sin2s[HH:P, sl], MULT
                )
                tcos = small.tile([P, SB], f32, tag="tcos")
                nc.vector.tensor_tensor(tcos[:], pq[:], cos2[:, sl], MULT)
                nc.vector.tensor_tensor(out_tile[:, sl], tcos[:], tsin[:], ADD)

        qT = [persist.tile([P, S], bf16, name=f"qT{h}", tag=f"qT{h}") for h in range(QH)]
        kT = persist.tile([P, S], bf16)
        for h in range(QH):
            proj_qk(wq_sb[h], qT[h])
        proj_qk(wk_sb, kT)

        # ---------------- v projection (v' with ones column) -------------
        # layout [P, NT, H+4]: v[t = tt*P + p, 0:H], v'[t, H] = 1
        VW = H + 4
        vp = persist.tile([P, NT, VW], bf16)
        for tt in range(NT):
            pv = ps_sc.tile([P, P], f32, tag="sc", name="pv")
            for dk in range(DK):
                nc.tensor.matmul(
                    pv[:],
                    xT[:, tt, dk * P : (dk + 1) * P],
                    wv_sb[:, dk, :],
                    start=(dk == 0),
                    stop=(dk == DK - 1),
                )
            nc.vector.tensor_copy(vp[:, tt, 0:H], pv[:])
            nc.gpsimd.memset(vp[:, tt, H : H + 1], 1.0)

        # ---------------- attention + fused O projection, per sq block ----
        # O-projection of block sb-1 is interleaved between the attention
        # heads of block sb so its PSUM-evict waits don't stall the PE queue.
        def oproj_tile(sb, attnT_blk, r2):
            st = RB * sb + r2
            for db in range(D // SB):
                po = ps_sc.tile([P, SB], f32, tag="sc", name="po")
                for h in range(QH):
                    nc.tensor.matmul(
                        po[:],
                        attnT_blk[h][:, r2 * P : (r2 + 1) * P],
                        wo_sb[h][:, db * SB : (db + 1) * SB],
                        start=(h == 0),
                        stop=(h == QH - 1),
                    )
                ob = ob_pool.tile([P, SB], f32, tag="ob")
                nc.vector.tensor_copy(ob[:], po[:])
                nc.sync.dma_start(
                    o_d[st * P : (st + 1) * P, db * SB : (db + 1) * SB], ob[:]
                )

        def attention_head(sb, h, attnT):
            pav = [
                ps_av.tile(
                    [P, H + 1], f32, name=f"pav{r}", tag=f"av{r}", bufs=1
                )[:]
                for r in range(RB)
            ]
            ptr2 = ps_sc.tile([P, SB], bf16, tag="sc", name="ptr2")
            ans = [None] * RB

            def finish_subtile(r2):
                rec = small.tile([P, 1], f32, tag="rec", bufs=4)
                nc.vector.reciprocal(rec[:], pav[r2][:, H : H + 1])
                an = small.tile([P, H], bf16, tag="an", bufs=4)
                nc.vector.tensor_scalar_mul(an[:], pav[r2][:, 0:H], rec[:])
                ans[r2] = an

            for tt in range(RB * (sb + 1)):
                pscore = ps_sc.tile([P, SB], f32, tag="sc", name="pscore")
                nc.tensor.matmul(
                    pscore[:],
                    kT[:, tt * P : (tt + 1) * P],
                    qT[h][:, sb * SB : (sb + 1) * SB],
                    start=True,
                    stop=True,
                )
                r = tt - RB * sb
                if r >= 0:
                    nc.vector.tensor_tensor(
                        pscore[:, r * P : (r + 1) * P],
                        pscore[:, r * P : (r + 1) * P],
                        mask[:],
                        ADD,
                    )
                pt = pt_pool.tile([P, SB], bf16, tag="pt")
                c0 = max(0, r) * P
                nc.scalar.activation(
                    pt[:, c0:SB], pscore[:, c0:SB], EXP, bias=exp_bias[:]
                )
                for r2 in range(max(0, r), RB):
                    q128 = RB * sb + r2
                    nc.tensor.matmul(
                        pav[r2],
                        pt[:, r2 * P : (r2 + 1) * P],
                        vp[:, tt, 0 : H + 1],
                        start=(tt == 0),
                        stop=(tt == q128),
                    )
                if r >= 0:
                    finish_subtile(r)
            for r2 in range(RB):
                nc.tensor.transpose(
                    ptr2[:, r2 * P : (r2 + 1) * P], ans[r2][:], ident[:]
                )
                sl2 = slice(r2 * P, (r2 + 1) * P)
                nc.scalar.copy(attnT[h][:, sl2], ptr2[:, sl2])

        prev = None
        for sb in range(NSB):
            attnT = [
                at_pool.tile(
                    [P, SB], bf16, name=f"attnT{h}", tag=f"attnT{h}", bufs=2
                )
                for h in range(QH)
            ]
            for h in range(QH):
                attention_head(sb, h, attnT)
                if prev is not None:
                    oproj_tile(sb - 1, prev, h)
            prev = attnT
        for r2 in range(RB):
            oproj_tile(NSB - 1, prev, r2)

    nc.compile()
    return nc


_NC_CACHE = {}


def _get_nc(key):
    if key not in _NC_CACHE:
        _NC_CACHE[key] = build_nc(*key)
    return _NC_CACHE[key]


def make_in_maps(x, positions, Wq, Wk, Wv, Wo, n_cores=8):
    B, S, D = x.shape
    Q, _, H = Wq.shape
    N = Wk.shape[0]
    groups = Q // N if N else 1
    gpb = n_cores // B  # head groups per batch (4)
    qh_per_core = Q // gpb
    assert qh_per_core * gpb == Q
    scale = np.float32(1.0 / math.sqrt(H))
    in_maps = []
    for c in range(n_cores):
        b = c // gpb
        g = c % gpb
        qh0 = g * qh_per_core
        kvh = qh0 // groups
        in_maps.append(
            {
                "x": np.ascontiguousarray(x[b]),
                "positions": positions,
                "wq": np.ascontiguousarray(Wq[qh0 : qh0 + qh_per_core] * scale),
                "wk": np.ascontiguousarray(Wk[kvh]),
                "wv": np.ascontiguousarray(Wv[kvh]),
                "wo": np.ascontiguousarray(Wo[qh0 : qh0 + qh_per_core]),
            }
        )
    return in_maps, gpb, qh_per_core


def kernel(x, positions, Wq, Wk, Wv, Wo):
    """Full inputs -> full output.  x [B,S,D] f32, positions [S] i32,
    Wq [Q,D,H], Wk/Wv [N,D,H], Wo [Q,H,D].  Returns [B,S,D] f32."""
    from concourse.bass_utils import run_bass_kernel_spmd

    x = np.ascontiguousarray(np.asarray(x, dtype=np.float32))
    positions = np.ascontiguousarray(np.asarray(positions, dtype=np.int32))
    Wq = np.asarray(Wq, dtype=np.float32)
    Wk = np.asarray(Wk, dtype=np.float32)
    Wv = np.asarray(Wv, dtype=np.float32)
    Wo = np.asarray(Wo, dtype=np.float32)

    B, S, D = x.shape
    Q, _, H = Wq.shape
    n_cores = 8
    in_maps, gpb, qh_per_core = make_in_maps(x, positions, Wq, Wk, Wv, Wo, n_cores)

    nc = _get_nc((S, D, qh_per_core, H))
    res = run_bass_kernel_spmd(nc, in_maps, core_ids=list(range(n_cores)))
    out = np.zeros((B, S, D), dtype=np.float32)
    for c in range(n_cores):
        out[c // gpb] += res.results[c]["o"]
    return out


# revision 39
# speedup vs baseline: 1.1658x; 1.0015x over previous
"""Trainium2 Bass kernel for GQA attention (B=2, S=2048, D=2048, 16 q-heads,
4 kv-heads, head_dim=128, RoPE, causal) sharded over 8 NeuronCores.

Sharding: core c handles batch b = c//4 and q-head group g = c%4
(q-heads 4g..4g+3, which share kv-head g).  Each core computes a partial
output o_part[b] = sum_{its heads} attn_head @ Wo_head; the host sums the
4 partials per batch.
"""

import sys

sys.path.insert(0, "/opt/trn_rl_repo")

import math

import numpy as np

P = 128
NEG = -1.0e9
EXP_BIAS = -8.0  # exp(s - 8): cancels in softmax normalization, avoids overflow


def build_nc(S=2048, D=2048, QH=4, H=128, theta=10000.0):
    """Build the per-core Bass graph.

    Per-core problem: x [S, D] f32, positions [S] i32,
    wq [QH, D, H] f32 (pre-scaled by 1/sqrt(H)), wk/wv [D, H] f32,
    wo [QH, H, D] f32  ->  o [S, D] f32 (partial over heads).
    """
    import concourse.bacc as bacc
    import concourse.mybir as mybir
    from concourse import tile
    from concourse.masks import make_identity

    f32 = mybir.dt.float32
    bf16 = mybir.dt.bfloat16
    i32 = mybir.dt.int32
    ADD = mybir.AluOpType.add
    MULT = mybir.AluOpType.mult
    EXP = mybir.ActivationFunctionType.Exp
    SIN = mybir.ActivationFunctionType.Sin

    assert H == P
    HH = H // 2  # 64
    DK = D // P  # d-chunks
    NSQ = S // P  # s-tiles
    SB = min(512, S)  # sq block width
    NSB = S // SB  # sq blocks
    RB = SB // P  # sq subtiles per block
    NT = S // P  # t tiles
    TRG = min(512, D)  # transpose psum group width
    CS = min(256, S)  # rope chunk width

    nc = bacc.Bacc(None, target_bir_lowering=False)

    x_d = nc.declare_dram_parameter("x", [S, D], f32, isOutput=False)
    pos_d = nc.declare_dram_parameter("positions", [S], i32, isOutput=False)
    wq_d = nc.declare_dram_parameter("wq", [QH, D, H], f32, isOutput=False)
    wk_d = nc.declare_dram_parameter("wk", [D, H], f32, isOutput=False)
    wv_d = nc.declare_dram_parameter("wv", [D, H], f32, isOutput=False)
    wo_d = nc.declare_dram_parameter("wo", [QH, H, D], f32, isOutput=False)
    o_d = nc.declare_dram_parameter("o", [S, D], f32, isOutput=True)

    from contextlib import ExitStack

    with tile.TileContext(nc) as tc, ExitStack() as es:
        # ---------------- pools ----------------
        const = es.enter_context(tc.tile_pool(name="const", bufs=1))
        stage = es.enter_context(tc.tile_pool(name="stage", bufs=2))
        persist = es.enter_context(tc.tile_pool(name="persist", bufs=1))
        small = es.enter_context(tc.tile_pool(name="small", bufs=2))
        pt_pool = es.enter_context(tc.tile_pool(name="pt", bufs=6))
        ob_pool = es.enter_context(tc.tile_pool(name="ob", bufs=2))
        at_pool = es.enter_context(tc.tile_pool(name="at", bufs=1))
        # PSUM: "sc" (x-transpose groups, scores, O proj) 3 banks,
        # ptr2 (attn transpose) 1 bank, AV accumulators 4 banks.
        ps_sc = es.enter_context(tc.tile_pool(name="ps_sc", bufs=4, space="PSUM"))
        ps_av = es.enter_context(tc.tile_pool(name="ps_av", bufs=1, space="PSUM"))

        # ---------------- constants ----------------
        identf = const.tile([P, P], f32)
        make_identity(nc, identf)
        ident = const.tile([P, P], bf16)
        make_identity(nc, ident)

        exp_bias = const.tile([P, 1], f32)
        nc.gpsimd.memset(exp_bias[:], EXP_BIAS)

        # causal additive mask for the diagonal [P, P] sub-block of a
        # scoresT tile: keep (0) where y >= x, else NEG.
        mask = const.tile([P, P], f32)
        nc.gpsimd.memset(mask[:], 0.0)
        nc.gpsimd.affine_select(
            out=mask[:],
            in_=mask[:],
            compare_op=mybir.AluOpType.is_ge,
            fill=NEG,
            base=0,
            pattern=[[1, P]],
            channel_multiplier=-1,
        )

        # ---------------- x load + transpose (f32 in PE, cast on evict) --
        # xT layout [p, st, dk*P + u] : element x(s = st*P + u, d = dk*P + p)
        xT = persist.tile([P, NSQ, D], bf16)
        for st in range(NSQ):
            xf = stage.tile([P, D], f32, tag="xf", name="xf", bufs=3)
            for g in range(D // TRG):
                nc.sync.dma_start(
                    xf[:, g * TRG : (g + 1) * TRG],
                    x_d[st * P : (st + 1) * P, g * TRG : (g + 1) * TRG],
                )
            for g in range(D // TRG):
                ptr = ps_sc.tile([P, TRG], f32, tag="sc", name="ptr")
                for j in range(TRG // P):
                    dk = (TRG // P) * g + j
                    nc.tensor.transpose(
                        ptr[:, j * P : (j + 1) * P],
                        xf[:, dk * P : (dk + 1) * P],
                        identf[:],
                    )
                if (st + g) % 2 == 0:
                    nc.scalar.copy(xT[:, st, g * TRG : (g + 1) * TRG], ptr[:])
                else:
                    nc.vector.tensor_copy(
                        xT[:, st, g * TRG : (g + 1) * TRG], ptr[:]
                    )

        # ---------------- rope tables (emitted first: DVE chain runs
        # while x DMAs stream on the sync queue) ----------------
        # inv_ts[i] = theta ** (-2 i / H), i in [0, HH)
        iot = const.tile([HH, 1], i32)
        nc.gpsimd.iota(iot[:], pattern=[[0, 1]], base=0, channel_multiplier=1)
        iotf = const.tile([HH, 1], f32)
        nc.vector.tensor_copy(iotf[:], iot[:])
        inv_ts = const.tile([HH, 1], f32)
        nc.scalar.activation(
            inv_ts[:], iotf[:], EXP, scale=-2.0 * math.log(theta) / H
        )

        TWO_PI = float(np.float32(2.0 * math.pi))
        PI = float(np.float32(math.pi))

        # cos2[h] = cos(angle_{h mod HH}); sin2s[h<HH] = -sin, sin2s[h>=HH] = +sin
        cos2 = persist.tile([P, S], f32)
        sin2s = persist.tile([P, S], f32)

        for c0 in range(0, S, CS):
            sl = slice(c0, c0 + CS)
            posi = const.tile([1, CS], i32, tag="rr_pi", name="posi")
            nc.gpsimd.dma_start(
                posi[:], pos_d.rearrange("(a s) -> a s", a=1)[:, sl]
            )
            posf = const.tile([1, CS], f32, tag="rr_pf", name="posf")
            nc.vector.tensor_copy(posf[:], posi[:])
            pb = const.tile([HH, CS], f32, tag="rr_pb", name="pb")
            nc.gpsimd.partition_broadcast(pb[:], posf[:])
            ang = const.tile([HH, CS], f32, tag="rr_ang", name="ang")
            nc.vector.tensor_scalar_mul(ang[:], pb[:], inv_ts[:])

            def sin_reduced(dst, phase):
                # dst = sin(ang + phase).  k = int-cast((ang+phase)/2pi):
                # trunc (sim) gives red in [0, 2pi); round (hw) gives
                # [-pi, pi].  One conditional -2pi brings both to [-pi, pi].
                if phase != 0.0:
                    a = const.tile([HH, CS], f32, tag="rr_a", name="a", bufs=1)
                    nc.vector.tensor_scalar_add(a[:], ang[:], phase)
                else:
                    a = ang
                t = const.tile([HH, CS], f32, tag="rr_t", name="t", bufs=1)
                nc.vector.tensor_scalar_mul(t[:], a[:], 1.0 / TWO_PI)
                ki = const.tile([HH, CS], i32, tag="rr_ki", name="ki", bufs=1)
                nc.vector.tensor_copy(ki[:], t[:])
                kf = const.tile([HH, CS], f32, tag="rr_kf", name="kf", bufs=1)
                nc.vector.tensor_copy(kf[:], ki[:])
                red = const.tile([HH, CS], f32, tag="rr_red", name="red", bufs=1)
                nc.vector.scalar_tensor_tensor(
                    red[:], kf[:], -TWO_PI, a[:], MULT, ADD
                )
                cc = const.tile([HH, CS], f32, tag="rr_c", name="cc", bufs=1)
                nc.vector.tensor_scalar(
                    cc[:], red[:], PI, None, op0=mybir.AluOpType.is_gt
                )
                nc.vector.scalar_tensor_tensor(
                    red[:], cc[:], -TWO_PI, red[:], MULT, ADD
                )
                nc.scalar.activation(dst[:], red[:], SIN)

            sin_reduced(cos2[0:HH, sl], float(np.float32(math.pi / 2.0)))
            sin_reduced(sin2s[HH:P, sl], 0.0)  # +sin in hi half

        nc.vector.tensor_copy(cos2[HH:P, :], cos2[0:HH, :])
        nc.vector.tensor_scalar_mul(sin2s[0:HH, :], sin2s[HH:P, :], -1.0)

        # ---------------- weights: load f32, cast to bf16 ----------------
        # layout [p, dk, h]: element (d = dk*P + p, h)
        def load_w_dh(dram_ap, name):  # dram [D, H] -> sbuf bf16 [P, DK, H]
            wf = stage.tile([P, DK * H], f32, tag="xf", name="wf", bufs=3)
            nc.gpsimd.dma_start(
                wf[:].rearrange("p (k h) -> p k h", k=DK),
                dram_ap.rearrange("(k p) h -> p k h", p=P),
            )
            wb = persist.tile([P, DK, H], bf16, name=name, tag=name)
            nc.scalar.copy(wb[:], wf[:].rearrange("p (k h) -> p k h", k=DK))
            return wb

        wq_sb = [load_w_dh(wq_d[h], f"wq{h}") for h in range(QH)]
        wk_sb = load_w_dh(wk_d, "wk")
        wv_sb = load_w_dh(wv_d, "wv")

        # wo: [H, D] per head -> sbuf bf16 [P, D] (partition = h)
        wo_sb = []
        for h in range(QH):
            wf = stage.tile([P, D], f32, tag="xf", name="wf", bufs=3)
            nc.gpsimd.dma_start(wf[:], wo_d[h])
            wb = persist.tile([P, D], bf16, name=f"wo{h}", tag=f"wo{h}")
            nc.scalar.copy(wb[:], wf[:])
            wo_sb.append(wb)

        # ---------------- q/k projections with rope ----------------
        def proj_qk(w_sb, out_tile):
            for sb in range(NSB):
                pq = ps_sc.tile([P, SB], f32, tag="sc", name="pq")
                for dk in range(DK):
                    nc.tensor.matmul(
                        pq[:],
                        w_sb[:, dk, :],
                        xT[:, sb * RB : (sb + 1) * RB, dk * P : (dk + 1) * P],
                        start=(dk == 0),
                        stop=(dk == DK - 1),
                    )
                sl = slice(sb * SB, (sb + 1) * SB)
                # rope: out = pq * cos2 + rot(pq) * sin2s
                tsin = small.tile([P, SB], f32, tag="tsin")
                nc.vector.tensor_tensor(
                    tsin[0:HH, :], pq[HH:P, :], sin2s[0:HH, sl], MULT
                )
                nc.vector.tensor_tensor(
                    tsin[HH:P, :], pq[0:HH, :], sin2s[HH:P, sl], MULT
                )
                tcos = small.tile([P, SB], f32, tag="tcos")
                nc.vector.tensor_tensor(tcos[:], pq[:], cos2[:, sl], MULT)
                nc.vector.tensor_tensor(out_tile[:, sl], tcos[:], tsin[:], ADD)

        qT = [persist.tile([P, S], bf16, name=f"qT{h}", tag=f"qT{h}") for h in range(QH)]
        kT = persist.tile([P, S], bf16)
        for h in range(QH):
            proj_qk(wq_sb[h], qT[h])
        proj_qk(wk_sb, kT)

        # ---------------- v projection (v' with ones column) -------------
        # layout [P, NT, H+4]: v[t = tt*P + p, 0:H], v'[t, H] = 1
        VW = H + 4
        vp = persist.tile([P, NT, VW], bf16)
        for tt in range(NT):
            pv = ps_sc.tile([P, P], f32, tag="sc", name="pv")
            for dk in range(DK):
                nc.tensor.matmul(
                    pv[:],
                    xT[:, tt, dk * P : (dk + 1) * P],
                    wv_sb[:, dk, :],
                    start=(dk == 0),
                    stop=(dk == DK - 1),
                )
            nc.vector.tensor_copy(vp[:, tt, 0:H], pv[:])
            nc.gpsimd.memset(vp[:, tt, H : H + 1], 1.0)

        # ---------------- attention + fused O projection, per sq block ----
        # O-projection of block sb-1 is interleaved between the attention
        # heads of block sb so its PSUM-evict waits don't stall the PE queue.
        def oproj_tile(sb, attnT_blk, r2):
            st = RB * sb + r2
            for db in range(D // SB):
                po = ps_sc.tile([P, SB], f32, tag="sc", name="po")
                for h in range(QH):
                    nc.tensor.matmul(
                        po[:],
                        attnT_blk[h][:, r2 * P : (r2 + 1) * P],
                        wo_sb[h][:, db * SB : (db + 1) * SB],
                        start=(h == 0),
                        stop=(h == QH - 1),
                    )
                ob = ob_pool.tile([P, SB], f32, tag="ob")
                nc.vector.tensor_copy(ob[:], po[:])
                nc.sync.dma_start(
                    o_d[st * P : (st + 1) * P, db * SB : (db + 1) * SB], ob[:]
                )

        def attention_head(sb, h, attnT):
            pav = [
                ps_av.tile(
                    [P, H + 1], f32, name=f"pav{r}", tag=f"av{r}", bufs=1
                )[:]
                for r in range(RB)
            ]
            ptr2 = ps_sc.tile([P, SB], bf16, tag="sc", name="ptr2")
            ans = [None] * RB

            def finish_subtile(r2):
                rec = small.tile([P, 1], f32, tag="rec", bufs=4)
                nc.vector.reciprocal(rec[:], pav[r2][:, H : H + 1])
                an = small.tile([P, H], bf16, tag="an", bufs=4)
                nc.vector.tensor_scalar_mul(an[:], pav[r2][:, 0:H], rec[:])
                ans[r2] = an

            def emit_transpose(r2):
                nc.tensor.transpose(
                    ptr2[:, r2 * P : (r2 + 1) * P], ans[r2][:], ident[:]
                )
                sl2 = slice(r2 * P, (r2 + 1) * P)
                nc.scalar.copy(attnT[h][:, sl2], ptr2[:, sl2])

            for tt in range(RB * (sb + 1)):
                pscore = ps_sc.tile([P, SB], f32, tag="sc", name="pscore")
                nc.tensor.matmul(
                    pscore[:],
                    kT[:, tt * P : (tt + 1) * P],
                    qT[h][:, sb * SB : (sb + 1) * SB],
                    start=True,
                    stop=True,
                )
                r = tt - RB * sb
                if r >= 0:
                    nc.vector.tensor_tensor(
                        pscore[:, r * P : (r + 1) * P],
                        pscore[:, r * P : (r + 1) * P],
                        mask[:],
                        ADD,
                    )
                pt = pt_pool.tile([P, SB], bf16, tag="pt")
                c0 = max(0, r) * P
                nc.scalar.activation(
                    pt[:, c0:SB], pscore[:, c0:SB], EXP, bias=exp_bias[:]
                )
                for r2 in range(max(0, r), RB):
                    q128 = RB * sb + r2
                    nc.tensor.matmul(
                        pav[r2],
                        pt[:, r2 * P : (r2 + 1) * P],
                        vp[:, tt, 0 : H + 1],
                        start=(tt == 0),
                        stop=(tt == q128),
                    )
                if r >= 0:
                    finish_subtile(r)
                if r >= 1:
                    emit_transpose(r - 1)
            emit_transpose(RB - 1)

        prev = None
        for sb in range(NSB):
            attnT = [
                at_pool.tile(
                    [P, SB], bf16, name=f"attnT{h}", tag=f"attnT{h}", bufs=2
                )
                for h in range(QH)
            ]
            for h in range(QH):
                attention_head(sb, h, attnT)
                if prev is not None:
                    oproj_tile(sb - 1, prev, h)
            prev = attnT
        for r2 in range(RB):
            oproj_tile(NSB - 1, prev, r2)

    nc.compile()
    return nc


_NC_CACHE = {}


def _get_nc(key):
    if key not in _NC_CACHE:
        _NC_CACHE[key] = build_nc(*key)
    return _NC_CACHE[key]


def make_in_maps(x, positions, Wq, Wk, Wv, Wo, n_cores=8):
    B, S, D = x.shape
    Q, _, H = Wq.shape
    N = Wk.shape[0]
    groups = Q // N if N else 1
    gpb = n_cores // B  # head groups per batch (4)
    qh_per_core = Q // gpb
    assert qh_per_core * gpb == Q
    scale = np.float32(1.0 / math.sqrt(H))
    in_maps = []
    for c in range(n_cores):
        b = c // gpb
        g = c % gpb
        qh0 = g * qh_per_core
        kvh = qh0 // groups
        in_maps.append(
            {
                "x": np.ascontiguousarray(x[b]),
                "positions": positions,
                "wq": np.ascontiguousarray(Wq[qh0 : qh0 + qh_per_core] * scale),
                "wk": np.ascontiguousarray(Wk[kvh]),
                "wv": np.ascontiguousarray(Wv[kvh]),
                "wo": np.ascontiguousarray(Wo[qh0 : qh0 + qh_per_core]),
            }
        )
    return in_maps, gpb, qh_per_core


def kernel(x, positions, Wq, Wk, Wv, Wo):
    """Full inputs -> full output.  x [B,S,D] f32, positions [S] i32,
    Wq [Q,D,H], Wk/Wv [N,D,H], Wo [Q,H,D].  Returns [B,S,D] f32."""
    from concourse.bass_utils import run_bass_kernel_spmd

    x = np.ascontiguousarray(np.asarray(x, dtype=np.float32))
    positions = np.ascontiguousarray(np.asarray(positions, dtype=np.int32))
    Wq = np.asarray(Wq, dtype=np.float32)
    Wk = np.asarray(Wk, dtype=np.float32)
    Wv = np.asarray(Wv, dtype=np.float32)
    Wo = np.asarray(Wo, dtype=np.float32)

    B, S, D = x.shape
    Q, _, H = Wq.shape
    n_cores = 8
    in_maps, gpb, qh_per_core = make_in_maps(x, positions, Wq, Wk, Wv, Wo, n_cores)

    nc = _get_nc((S, D, qh_per_core, H))
    res = run_bass_kernel_spmd(nc, in_maps, core_ids=list(range(n_cores)))
    out = np.zeros((B, S, D), dtype=np.float32)
    for c in range(n_cores):
        out[c // gpb] += res.results[c]["o"]
    return out
